# revision 1
# baseline (speedup 1.0000x reference)
"""Trainium2 Bass kernel for nn_LinearLatentKernel_84834194031187.

Computes, for x:[B,S,D], W_qkv:[3D,D], W_gate:[D,D] (fp32):
    qkv = x @ W_qkv.T + b_qkv ; q,k,v = split(qkv)
    kv_state = cumsum(k*v, axis=seq)
    out = q * kv_state * sigmoid(x @ W_gate.T + b_gate)

Sharding: 8 cores = (batch b in 0..3) x (channel half h in 0..1). Each core
handles x[b] [S,D] against a host-pretransposed weight slice W^T [D, 4*H]
(q,k,v,gate halves of H=512 channels each) and produces out[b,:,h*H:(h+1)*H].

Per core, seq is processed in 32 blocks of 128 rows (partition dim = seq):
  - x block [128, D] is PE-transposed into x^T tiles [d=128, s=128] (8 per block)
  - q/k/v/g chunks [128, 512] accumulate in PSUM over 8 contraction tiles,
    using float32r matmuls (TF32-like 11-bit mantissa, 4x faster than fp32)
  - kv = k*v; block-cumsum via matmul with an upper-triangular ones lhsT;
    the running carry (a [1,512] row) is broadcast-added with a rank-1 matmul
    and updated via a column-sum matmul (keeps the value on partition 0,
    since compute engines cannot move data across partitions)
  - out = (q * sigmoid(g)) * kv_state, streamed back to DRAM
"""

import numpy as np

import concourse.bass as bass
import concourse.bacc as bacc
import concourse.tile as tile
import concourse.mybir as mybir
from concourse.bass_utils import run_bass_kernel_spmd

B, S, D = 4, 4096, 1024
H = 512          # channels per core (half of D)
P = 128
NBLK = S // P    # 32 seq blocks
KT = D // P      # 8 contraction tiles

f32 = mybir.dt.float32
f32r = mybir.dt.float32r
bf16 = mybir.dt.bfloat16
f16 = mybir.dt.float16

# Projection-matmul operand dtype. 2-byte dtypes get fast weight loads
# (the LDWEIGHTS stream is the PE bottleneck at fp32r); fp16 keeps a
# 10-bit mantissa (~fp32r accuracy) while bf16 drops to 8 bits.
# All values are O(1) so fp16 range is safe. The cumsum chain stays fp32r.
PROJ_DT = "f16"

_NC_CACHE = {}


def _build(with_bias: bool, proj: str = PROJ_DT):
    proj_dt = {"f16": f16, "bf16": bf16, "f32r": f32r}[proj]
    nc = bacc.Bacc("TRN2", target_bir_lowering=False)

    # x arrives pre-cast to the projection dtype (host-side rounding is
    # identical to the rounding the x^T copies would apply on device)
    x_d = nc.dram_tensor("x", [S, D], proj_dt, kind="ExternalInput")
    wt_d = nc.dram_tensor("wt", [D, 4 * H], proj_dt, kind="ExternalInput")
    idn_d = nc.dram_tensor("idn", [P, P], proj_dt, kind="ExternalInput")
    tri_d = nc.dram_tensor("tri", [P, P], f32r, kind="ExternalInput")
    onescol_d = nc.dram_tensor("onescol", [P, 1], f32r, kind="ExternalInput")
    onesrow_d = nc.dram_tensor("onesrow", [1, P], f32r, kind="ExternalInput")
    if with_bias:
        bias_d = nc.dram_tensor("bias", [1, 4 * H], f32r, kind="ExternalInput")
    out_d = nc.dram_tensor("out", [S, H], f32, kind="ExternalOutput")

    with tile.TileContext(nc) as tc:
        with (
            tc.tile_pool(name="consts", bufs=1) as consts,
            tc.tile_pool(name="xin", bufs=3) as xin,
            tc.tile_pool(name="xtp", bufs=2) as xtp,
            tc.tile_pool(name="work", bufs=2) as work,
            tc.tile_pool(name="outp", bufs=3) as outp,
            tc.tile_pool(name="pmm", bufs=1, space="PSUM") as pmm,
            tc.tile_pool(name="pcs_pool", bufs=1, space="PSUM") as pcs_pool,
            tc.tile_pool(name="ptr", bufs=2, space="PSUM") as ptr,
            tc.tile_pool(name="pcarry", bufs=1, space="PSUM") as pcarry,
        ):
            # x block 0 first so PE transposes can start before W^T lands
            xb0 = xin.tile([P, D], proj_dt, tag="xb", name="xb0")
            nc.sync.dma_start(xb0[:], x_d[0:P, :])
            idn_sb = consts.tile([P, P], proj_dt, tag="idn")
            nc.sync.dma_start(idn_sb[:], idn_d[:])
            # W^T split per contraction tile: first matmuls only wait on wt[kt=0]
            wt_sb = consts.tile([P, KT, 4 * H], proj_dt, tag="wt")
            for kt in range(KT):
                nc.sync.dma_start(wt_sb[:, kt, :], wt_d[kt * P:(kt + 1) * P, :])
            tri_sb = consts.tile([P, P], f32r, tag="tri")
            nc.sync.dma_start(tri_sb[:], tri_d[:])
            onescol_sb = consts.tile([P, 1], f32r, tag="onescol")
            nc.sync.dma_start(onescol_sb[:], onescol_d[:])
            onesrow_sb = consts.tile([1, P], f32r, tag="onesrow")
            nc.sync.dma_start(onesrow_sb[:], onesrow_d[:])
            carry_sb = consts.tile([1, H], f32r, tag="carry")
            if with_bias:
                bias_sb = consts.tile([1, 4 * H], f32r, tag="bias")
                nc.sync.dma_start(bias_sb[:], bias_d[:])

            # running column-sum of kv accumulates here across all blocks
            pca = pcarry.tile([1, H], f32, tag="pca", name="pca")

            for i in range(NBLK):
                if i == 0:
                    xb = xb0
                else:
                    xb = xin.tile([P, D], proj_dt, tag="xb")
                    nc.sync.dma_start(xb[:], x_d[i * P:(i + 1) * P, :])

                xT = xtp.tile([P, KT, P], proj_dt, tag="xT")
                for kt in range(KT):
                    pt = ptr.tile([P, P], proj_dt, tag="pt")
                    nc.tensor.transpose(pt[:], xb[:, kt * P:(kt + 1) * P], idn_sb[:])
                    nc.any.tensor_copy(out=xT[:, kt, :], in_=pt[:])

                ps = [
                    pmm.tile([P, H], f32, tag=f"ps{c}", name=f"ps{c}")
                    for c in range(4)
                ]
                for kt in range(KT):
                    for c in range(4):
                        nc.tensor.matmul(
                            ps[c][:], xT[:, kt, :], wt_sb[:, kt, c * H:(c + 1) * H],
                            start=(kt == 0), stop=(kt == KT - 1 and not with_bias),
                        )
                if with_bias:
                    for c in range(4):
                        nc.tensor.matmul(
                            ps[c][:], onesrow_sb[:], bias_sb[:, c * H:(c + 1) * H],
                            start=False, stop=True,
                        )

                g_sb = work.tile([P, H], f32, tag="g")
                nc.scalar.activation(
                    g_sb[:], ps[3][:], mybir.ActivationFunctionType.Sigmoid
                )
                k_sb = work.tile([P, H], f32, tag="k")
                nc.any.tensor_copy(out=k_sb[:], in_=ps[1][:])
                kv_sb = work.tile([P, H], f32r, tag="kv")
                nc.vector.tensor_mul(out=kv_sb[:], in0=k_sb[:], in1=ps[2][:])

                # block cumsum (rows) + running-carry broadcast, all on PE
                pcs = pcs_pool.tile([P, H], f32, tag="pcs")
                nc.tensor.matmul(pcs[:], tri_sb[:], kv_sb[:],
                                 start=True, stop=(i == 0))
                if i > 0:
                    nc.tensor.matmul(pcs[:], onesrow_sb[:], carry_sb[:],
                                     start=False, stop=True)

                if i < NBLK - 1:
                    # pca accumulates colsum(kv) across blocks; its value after
                    # block i is the carry for block i+1
                    nc.tensor.matmul(pca[:], onescol_sb[:], kv_sb[:],
                                     start=(i == 0), stop=(i == NBLK - 2))
                    nc.any.tensor_copy(out=carry_sb[:], in_=pca[:])

                qg_sb = work.tile([P, H], f32, tag="qg")
                nc.vector.tensor_mul(out=qg_sb[:], in0=g_sb[:], in1=ps[0][:])
                ob = outp.tile([P, H], f32, tag="ob")
                nc.vector.tensor_mul(out=ob[:], in0=qg_sb[:], in1=pcs[:])
                nc.sync.dma_start(out_d[i * P:(i + 1) * P, :], ob[:])

    nc.compile()
    return nc


def _get_nc(with_bias: bool):
    if with_bias not in _NC_CACHE:
        _NC_CACHE[with_bias] = _build(with_bias)
    return _NC_CACHE[with_bias]


def _proj_np_dtype():
    if PROJ_DT == "bf16":
        import ml_dtypes
        return ml_dtypes.bfloat16
    if PROJ_DT == "f16":
        return np.float16
    return np.float32


def _prep_in_maps(x, W_qkv, b_qkv, W_gate, b_gate, with_bias):
    pdt = _proj_np_dtype()
    x = np.ascontiguousarray(np.asarray(x, dtype=np.float32)).astype(pdt)
    W_qkv = np.asarray(W_qkv, dtype=np.float32)
    W_gate = np.asarray(W_gate, dtype=np.float32)

    consts = {
        "idn": np.eye(P, dtype=pdt),
        "tri": np.triu(np.ones((P, P), dtype=np.float32)),
        "onescol": np.ones((P, 1), dtype=np.float32),
        "onesrow": np.ones((1, P), dtype=np.float32),
    }

    wts, biases = [], []
    for h in range(2):
        sl = slice(h * H, (h + 1) * H)
        wt = np.concatenate(
            [W_qkv[sl], W_qkv[D + h * H:D + (h + 1) * H],
             W_qkv[2 * D + h * H:2 * D + (h + 1) * H], W_gate[sl]], axis=0
        ).T
        wts.append(np.ascontiguousarray(wt).astype(pdt))
        if with_bias:
            bq = np.asarray(b_qkv, dtype=np.float32)
            bg = np.asarray(b_gate, dtype=np.float32)
            biases.append(np.concatenate(
                [bq[sl], bq[D + h * H:D + (h + 1) * H],
                 bq[2 * D + h * H:2 * D + (h + 1) * H], bg[sl]]
            )[None, :].copy())

    in_maps = []
    for core in range(8):
        b, h = core // 2, core % 2
        m = {"x": x[b], "wt": wts[h], **consts}
        if with_bias:
            m["bias"] = biases[h]
        in_maps.append(m)
    return in_maps


def run(x, W_qkv, b_qkv, W_gate, b_gate, trace=False, **run_kwargs):
    with_bias = bool(np.any(np.asarray(b_qkv)) or np.any(np.asarray(b_gate)))
    nc = _get_nc(with_bias)
    in_maps = _prep_in_maps(x, W_qkv, b_qkv, W_gate, b_gate, with_bias)
    res = run_bass_kernel_spmd(nc, in_maps, list(range(8)), trace=trace, **run_kwargs)
    out = np.empty((B, S, D), dtype=np.float32)
    for core in range(8):
        b, h = core // 2, core % 2
        out[b, :, h * H:(h + 1) * H] = res.results[core]["out"]
    return out, res


def kernel(x, W_qkv, b_qkv, W_gate, b_gate):
    out, _ = run(x, W_qkv, b_qkv, W_gate, b_gate)
    return out



# revision 3
# speedup vs baseline: 1.2327x; 1.2327x over previous
"""Trainium2 Bass kernel for nn_LinearLatentKernel_84834194031187.

Computes, for x:[B,S,D], W_qkv:[3D,D], W_gate:[D,D] (fp32):
    qkv = x @ W_qkv.T + b_qkv ; q,k,v = split(qkv)
    kv_state = cumsum(k*v, axis=seq)
    out = q * kv_state * sigmoid(x @ W_gate.T + b_gate)

Sharding: 8 cores = (batch b in 0..3) x (channel half h in 0..1); each core
computes out[b, :, h*512:(h+1)*512].

Layout: everything is computed TRANSPOSED, [channel, seq], so that
  - the projection matmuls take host-pretransposed x^T [D,S] directly as the
    moving operand (no PE transposes at all): ps[c,s] = wt[d,c]^T @ x^T[d,s]
  - the cumsum along seq becomes a FREE-dim prefix scan on the Vector engine
    (tensor_tensor_scan, chained across 512-col chunks via initial=prev[:,-1:])
so the Tensor engine runs ONLY the 1024 projection matmuls (8 kt x 4 types x
4 groups x 8 seq chunks, each [128x128]x[128x512] fp16 = 512 cycles), which is
the compute floor. Channel dim per core = 512 -> 4 groups of 128 partitions;
groups alternate between two sets of 4 PSUM banks (8 banks total), and each
type's consumers (sigmoid / copy / kv-mul+scan / final muls) fire as soon as
that type's 8-matmul accumulation completes, so PSUM recycling never stalls
the PE. Output is written transposed [512, S] and untransposed on the host.
"""

import numpy as np

import concourse.bass as bass
import concourse.bacc as bacc
import concourse.tile as tile
import concourse.mybir as mybir
from concourse.bass_utils import run_bass_kernel_spmd

B, S, D = 4, 4096, 1024
H = 512          # output channels per core (half of D)
P = 128
KT = D // P      # 8 contraction tiles
FD = 512         # seq columns per chunk (= PSUM bank capacity in fp32)
NJ = S // FD     # 8 seq chunks
NG = H // P      # 4 channel groups of 128
C = 4 * H        # 2048 projection columns per core (g,k,v,q per group)

f32 = mybir.dt.float32
f16 = mybir.dt.float16

# types within a group, in matmul order: gate first (longest consumer chain
# start), then k, v (kv-mul + scan), then q (final muls + store)
T_G, T_K, T_V, T_Q = 0, 1, 2, 3

_NC_CACHE = {}


def _build(with_bias: bool):
    nc = bacc.Bacc("TRN2", target_bir_lowering=False)

    xT_d = nc.dram_tensor("xT", [D, S], f16, kind="ExternalInput")
    wt_d = nc.dram_tensor("wt", [D, C], f16, kind="ExternalInput")
    if with_bias:
        bias_d = nc.dram_tensor("bias", [P, 4 * NG], f32, kind="ExternalInput")
    out_d = nc.dram_tensor("out", [H, S], f32, kind="ExternalOutput")

    sig = mybir.ActivationFunctionType.Sigmoid
    ident = mybir.ActivationFunctionType.Identity
    mult = mybir.AluOpType.mult
    add = mybir.AluOpType.add

    with tile.TileContext(nc) as tc:
        with (
            tc.tile_pool(name="consts", bufs=1) as consts,
            tc.tile_pool(name="xin", bufs=2) as xin,
            tc.tile_pool(name="work", bufs=1) as work,
            tc.tile_pool(name="scanp", bufs=2) as scanp,
            tc.tile_pool(name="outp", bufs=3) as outp,
            tc.tile_pool(name="psp", bufs=1, space="PSUM") as psp,
        ):
            # chunk-0 x^T tiles first so the first matmul can start early
            xts0 = []
            for kt in range(KT):
                xt = xin.tile([P, FD], f16, tag=f"x{kt}")
                nc.sync.dma_start(xt[:], xT_d[kt * P:(kt + 1) * P, 0:FD])
                xts0.append(xt)
            wt_sb = consts.tile([P, KT, C], f16, tag="wt")
            for kt in range(KT):
                nc.sync.dma_start(wt_sb[:, kt, :], wt_d[kt * P:(kt + 1) * P, :])
            ones_sb = consts.tile([P, FD], f32, tag="ones")
            nc.vector.memset(ones_sb[:], 1.0)
            if with_bias:
                bias_sb = consts.tile([P, 4 * NG], f32, tag="bias")
                nc.sync.dma_start(bias_sb[:], bias_d[:])

            prev_scan = {}
            for j in range(NJ):
                if j == 0:
                    xts = xts0
                else:
                    xts = []
                    for kt in range(KT):
                        xt = xin.tile([P, FD], f16, tag=f"x{kt}")
                        nc.sync.dma_start(
                            xt[:], xT_d[kt * P:(kt + 1) * P, j * FD:(j + 1) * FD]
                        )
                        xts.append(xt)

                for i in range(NG):
                    par = i % 2
                    ps = [
                        psp.tile([P, FD], f32, tag=f"ps{t}_{par}",
                                 name=f"ps{t}_{par}")
                        for t in range(4)
                    ]
                    bcol = (
                        (lambda t: bias_sb[:, (i * 4 + t):(i * 4 + t) + 1])
                        if with_bias else None
                    )
                    g_sb = k_sb = kv_sb = sc = None
                    for t in range(4):
                        col0 = (i * 4 + t) * P
                        for kt in range(KT):
                            nc.tensor.matmul(
                                ps[t][:],
                                wt_sb[:, kt, col0:col0 + P],
                                xts[kt][:],
                                start=(kt == 0),
                                stop=(kt == KT - 1),
                            )
                        # consumers fire as soon as this type's accumulation
                        # is complete, staggered across the group's matmuls
                        if t == T_G:
                            g_sb = work.tile([P, FD], f32, tag=f"g{par}")
                            nc.scalar.activation(
                                g_sb[:], ps[T_G][:], sig,
                                bias=bcol(T_G) if with_bias else 0.0,
                            )
                        elif t == T_K:
                            k_sb = work.tile([P, FD], f32, tag=f"k{par}")
                            if with_bias:
                                nc.scalar.activation(
                                    k_sb[:], ps[T_K][:], ident, bias=bcol(T_K)
                                )
                            else:
                                nc.scalar.copy(k_sb[:], ps[T_K][:])
                        elif t == T_V:
                            kv_sb = work.tile([P, FD], f32, tag=f"kv{par}")
                            if with_bias:
                                nc.vector.scalar_tensor_tensor(
                                    kv_sb[:], ps[T_V][:], bcol(T_V), k_sb[:],
                                    add, mult,
                                )
                            else:
                                nc.vector.tensor_mul(
                                    out=kv_sb[:], in0=k_sb[:], in1=ps[T_V][:]
                                )
                            sc = scanp.tile([P, FD], f32, tag=f"scan{i}")
                            init = 0.0 if j == 0 else prev_scan[i][:, FD - 1:FD]
                            # state = (ones * state) + kv ; out[t] = state
                            nc.vector.tensor_tensor_scan(
                                sc[:], ones_sb[:], kv_sb[:], init, mult, add
                            )
                            prev_scan[i] = sc
                        else:  # T_Q
                            qg_sb = work.tile([P, FD], f32, tag=f"qg{par}")
                            if with_bias:
                                nc.vector.scalar_tensor_tensor(
                                    qg_sb[:], ps[T_Q][:], bcol(T_Q), g_sb[:],
                                    add, mult,
                                )
                            else:
                                nc.vector.tensor_mul(
                                    out=qg_sb[:], in0=g_sb[:], in1=ps[T_Q][:]
                                )
                            ob = outp.tile([P, FD], f32, tag="ob")
                            nc.vector.tensor_mul(
                                out=ob[:], in0=qg_sb[:], in1=sc[:]
                            )
                            nc.sync.dma_start(
                                out_d[i * P:(i + 1) * P, j * FD:(j + 1) * FD],
                                ob[:],
                            )

    nc.compile()
    return nc


def _get_nc(with_bias: bool):
    if with_bias not in _NC_CACHE:
        _NC_CACHE[with_bias] = _build(with_bias)
    return _NC_CACHE[with_bias]


def _prep_in_maps(x, W_qkv, b_qkv, W_gate, b_gate, with_bias):
    x = np.asarray(x, dtype=np.float32)
    W_qkv = np.asarray(W_qkv, dtype=np.float32)
    W_gate = np.asarray(W_gate, dtype=np.float32)

    xTs = [np.ascontiguousarray(x[b].T).astype(np.float16) for b in range(B)]

    # weight rows per (group, type): [g_i | k_i | v_i | q_i] blocks of 128
    wts, biases = [], []
    for h in range(2):
        blocks, bcols = [], []
        for i in range(NG):
            r0 = h * H + i * P
            rows = [
                W_gate[r0:r0 + P],
                W_qkv[D + r0:D + r0 + P],
                W_qkv[2 * D + r0:2 * D + r0 + P],
                W_qkv[r0:r0 + P],
            ]
            blocks.extend(rows)
            if with_bias:
                bq = np.asarray(b_qkv, dtype=np.float32)
                bg = np.asarray(b_gate, dtype=np.float32)
                bcols.extend([
                    bg[r0:r0 + P],
                    bq[D + r0:D + r0 + P],
                    bq[2 * D + r0:2 * D + r0 + P],
                    bq[r0:r0 + P],
                ])
        wt = np.concatenate(blocks, axis=0).T  # [1024, 2048]
        wts.append(np.ascontiguousarray(wt).astype(np.float16))
        if with_bias:
            biases.append(np.stack(bcols, axis=1).astype(np.float32))  # [128,16]

    in_maps = []
    for core in range(8):
        b, h = core // 2, core % 2
        m = {"xT": xTs[b], "wt": wts[h]}
        if with_bias:
            m["bias"] = biases[h]
        in_maps.append(m)
    return in_maps


def run(x, W_qkv, b_qkv, W_gate, b_gate, trace=False, **run_kwargs):
    with_bias = bool(np.any(np.asarray(b_qkv)) or np.any(np.asarray(b_gate)))
    nc = _get_nc(with_bias)
    in_maps = _prep_in_maps(x, W_qkv, b_qkv, W_gate, b_gate, with_bias)
    res = run_bass_kernel_spmd(nc, in_maps, list(range(8)), trace=trace, **run_kwargs)
    out = np.empty((B, S, D), dtype=np.float32)
    for core in range(8):
        b, h = core // 2, core % 2
        out[b, :, h * H:(h + 1) * H] = res.results[core]["out"].T
    return out, res


def kernel(x, W_qkv, b_qkv, W_gate, b_gate):
    out, _ = run(x, W_qkv, b_qkv, W_gate, b_gate)
    return out


# revision 9
# speedup vs baseline: 1.2363x; 1.0030x over previous
"""Trainium2 Bass kernel for nn_LinearLatentKernel_84834194031187.

Computes, for x:[B,S,D], W_qkv:[3D,D], W_gate:[D,D] (fp32):
    qkv = x @ W_qkv.T + b_qkv ; q,k,v = split(qkv)
    kv_state = cumsum(k*v, axis=seq)
    out = q * kv_state * sigmoid(x @ W_gate.T + b_gate)

Sharding: 8 cores = (batch b in 0..3) x (channel half h in 0..1); each core
computes out[b, :, h*512:(h+1)*512].

Layout: everything is computed TRANSPOSED, [channel, seq], so that
  - the projection matmuls take host-pretransposed x^T [D,S] directly as the
    moving operand (no PE transposes at all): ps[c,s] = wt[d,c]^T @ x^T[d,s]
  - the cumsum along seq becomes a FREE-dim prefix scan on the Vector engine
    (tensor_tensor_scan, chained across 512-col chunks via initial=prev[:,-1:])
so the Tensor engine runs ONLY the 1024 projection matmuls (8 kt x 4 types x
4 groups x 8 seq chunks, each [128x128]x[128x512] fp16 = 512 cycles), which is
the compute floor. Channel dim per core = 512 -> 4 groups of 128 partitions;
groups alternate between two sets of 4 PSUM banks (8 banks total), and each
type's consumers (sigmoid / copy / kv-mul+scan / final muls) fire as soon as
that type's 8-matmul accumulation completes, so PSUM recycling never stalls
the PE. Output is written transposed [512, S] and untransposed on the host.
"""

import numpy as np

import concourse.bass as bass
import concourse.bacc as bacc
import concourse.tile as tile
import concourse.mybir as mybir
from concourse.bass_utils import run_bass_kernel_spmd

B, S, D = 4, 4096, 1024
H = 512          # output channels per core (half of D)
P = 128
KT = D // P      # 8 contraction tiles
FD = 512         # seq columns per chunk (= PSUM bank capacity in fp32)
NJ = S // FD     # 8 seq chunks
NG = H // P      # 4 channel groups of 128
C = 4 * H        # 2048 projection columns per core (g,k,v,q per group)

f32 = mybir.dt.float32
f16 = mybir.dt.float16

# types within a group, in matmul order: gate first (longest consumer chain
# start), then k, v (kv-mul + scan), then q (final muls + store)
T_G, T_K, T_V, T_Q = 0, 1, 2, 3

_NC_CACHE = {}


def _build(with_bias: bool):
    nc = bacc.Bacc("TRN2", target_bir_lowering=False)

    # host supplies x^T and W^T pre-permuted to [partition, kt, col] so a
    # whole chunk/group moves in ONE dma (DMA issue on the sync queue is
    # ~600ns each, serial -- many small DMAs stall the PE at startup)
    xT_d = nc.dram_tensor("xT", [P, KT, S], f16, kind="ExternalInput")
    wt_d = nc.dram_tensor("wt", [P, KT, C], f16, kind="ExternalInput")
    if with_bias:
        bias_d = nc.dram_tensor("bias", [P, 4 * NG], f32, kind="ExternalInput")
    out_d = nc.dram_tensor("out", [H, S], f32, kind="ExternalOutput")

    sig = mybir.ActivationFunctionType.Sigmoid
    ident = mybir.ActivationFunctionType.Identity
    mult = mybir.AluOpType.mult
    add = mybir.AluOpType.add

    with tile.TileContext(nc) as tc:
        with (
            tc.tile_pool(name="consts", bufs=1) as consts,
            tc.tile_pool(name="xin", bufs=2) as xin,
            tc.tile_pool(name="work", bufs=1) as work,
            tc.tile_pool(name="scanp", bufs=2) as scanp,
            tc.tile_pool(name="outp", bufs=3) as outp,
            tc.tile_pool(name="psp", bufs=1, space="PSUM") as psp,
        ):
            # DMA order tuned for fast start: the (group0, kt0) weight slice
            # and (chunk0, kt0) x slice land first (first matmul's operands),
            # then the rest of group0/chunk0, then the remaining weight groups.
            GC = 4 * P  # 512 columns per group
            wt_sb = consts.tile([P, KT, C], f16, tag="wt")
            xts0 = xin.tile([P, KT, FD], f16, tag="x", name="xts0")
            nc.sync.dma_start(wt_sb[:, 0, 0:GC], wt_d[:, 0, 0:GC])
            nc.sync.dma_start(xts0[:, 0, :], xT_d[:, 0, 0:FD])
            nc.sync.dma_start(wt_sb[:, 1:KT, 0:GC], wt_d[:, 1:KT, 0:GC])
            nc.sync.dma_start(xts0[:, 1:KT, :], xT_d[:, 1:KT, 0:FD])
            for gi in range(1, NG):
                nc.sync.dma_start(
                    wt_sb[:, :, gi * GC:(gi + 1) * GC],
                    wt_d[:, :, gi * GC:(gi + 1) * GC],
                )
            ones_sb = consts.tile([P, FD], f32, tag="ones")
            nc.vector.memset(ones_sb[:], 1.0)
            if with_bias:
                bias_sb = consts.tile([P, 4 * NG], f32, tag="bias")
                nc.sync.dma_start(bias_sb[:], bias_d[:])

            prev_scan = {}
            for j in range(NJ):
                if j == 0:
                    xts = xts0
                else:
                    xts = xin.tile([P, KT, FD], f16, tag="x", name="xts")
                    nc.sync.dma_start(
                        xts[:], xT_d[:, :, j * FD:(j + 1) * FD]
                    )

                for i in range(NG):
                    par = i % 2
                    ps = [
                        psp.tile([P, FD], f32, tag=f"ps{t}_{par}",
                                 name=f"ps{t}_{par}")
                        for t in range(4)
                    ]
                    bcol = (
                        (lambda t: bias_sb[:, (i * 4 + t):(i * 4 + t) + 1])
                        if with_bias else None
                    )
                    g_sb = k_sb = kv_sb = sc = None
                    for t in range(4):
                        col0 = (i * 4 + t) * P
                        for kt in range(KT):
                            nc.tensor.matmul(
                                ps[t][:],
                                wt_sb[:, kt, col0:col0 + P],
                                xts[:, kt, :],
                                start=(kt == 0),
                                stop=(kt == KT - 1),
                            )
                        # consumers fire as soon as this type's accumulation
                        # is complete, staggered across the group's matmuls
                        if t == T_G:
                            g_sb = work.tile([P, FD], f32, tag=f"g{par}")
                            nc.scalar.activation(
                                g_sb[:], ps[T_G][:], sig,
                                bias=bcol(T_G) if with_bias else 0.0,
                            )
                        elif t == T_K:
                            k_sb = work.tile([P, FD], f32, tag=f"k{par}")
                            if with_bias:
                                nc.scalar.activation(
                                    k_sb[:], ps[T_K][:], ident, bias=bcol(T_K)
                                )
                            else:
                                nc.scalar.copy(k_sb[:], ps[T_K][:])
                        elif t == T_V:
                            kv_sb = work.tile([P, FD], f32, tag=f"kv{par}")
                            if with_bias:
                                nc.vector.scalar_tensor_tensor(
                                    kv_sb[:], ps[T_V][:], bcol(T_V), k_sb[:],
                                    add, mult,
                                )
                            else:
                                nc.vector.tensor_mul(
                                    out=kv_sb[:], in0=k_sb[:], in1=ps[T_V][:]
                                )
                            sc = scanp.tile([P, FD], f32, tag=f"scan{i}")
                            init = 0.0 if j == 0 else prev_scan[i][:, FD - 1:FD]
                            # state = (ones * state) + kv ; out[t] = state
                            nc.vector.tensor_tensor_scan(
                                sc[:], ones_sb[:], kv_sb[:], init, mult, add
                            )
                            prev_scan[i] = sc
                        else:  # T_Q
                            qg_sb = work.tile([P, FD], f32, tag=f"qg{par}")
                            if with_bias:
                                nc.vector.scalar_tensor_tensor(
                                    qg_sb[:], ps[T_Q][:], bcol(T_Q), g_sb[:],
                                    add, mult,
                                )
                            else:
                                nc.vector.tensor_mul(
                                    out=qg_sb[:], in0=g_sb[:], in1=ps[T_Q][:]
                                )
                            ob = outp.tile([P, FD], f32, tag="ob")
                            nc.vector.tensor_mul(
                                out=ob[:], in0=qg_sb[:], in1=sc[:]
                            )
                            nc.sync.dma_start(
                                out_d[i * P:(i + 1) * P, j * FD:(j + 1) * FD],
                                ob[:],
                            )

    nc.compile()
    return nc


def _get_nc(with_bias: bool):
    if with_bias not in _NC_CACHE:
        _NC_CACHE[with_bias] = _build(with_bias)
    return _NC_CACHE[with_bias]


def _prep_in_maps(x, W_qkv, b_qkv, W_gate, b_gate, with_bias):
    x = np.asarray(x, dtype=np.float32)
    W_qkv = np.asarray(W_qkv, dtype=np.float32)
    W_gate = np.asarray(W_gate, dtype=np.float32)

    # x^T as [partition p, kt, s]: element (p, kt, s) = x[s, kt*128+p]
    xTs = [
        np.ascontiguousarray(
            x[b].T.astype(np.float16).reshape(KT, P, S).transpose(1, 0, 2)
        )
        for b in range(B)
    ]

    # weight rows per (group, type): [g_i | k_i | v_i | q_i] blocks of 128
    wts, biases = [], []
    for h in range(2):
        blocks, bcols = [], []
        for i in range(NG):
            r0 = h * H + i * P
            rows = [
                W_gate[r0:r0 + P],
                W_qkv[D + r0:D + r0 + P],
                W_qkv[2 * D + r0:2 * D + r0 + P],
                W_qkv[r0:r0 + P],
            ]
            blocks.extend(rows)
            if with_bias:
                bq = np.asarray(b_qkv, dtype=np.float32)
                bg = np.asarray(b_gate, dtype=np.float32)
                bcols.extend([
                    bg[r0:r0 + P],
                    bq[D + r0:D + r0 + P],
                    bq[2 * D + r0:2 * D + r0 + P],
                    bq[r0:r0 + P],
                ])
        wt = np.concatenate(blocks, axis=0).T  # [1024, 2048]
        wts.append(np.ascontiguousarray(
            wt.astype(np.float16).reshape(KT, P, C).transpose(1, 0, 2)
        ))
        if with_bias:
            biases.append(np.stack(bcols, axis=1).astype(np.float32))  # [128,16]

    in_maps = []
    for core in range(8):
        b, h = core // 2, core % 2
        m = {"xT": xTs[b], "wt": wts[h]}
        if with_bias:
            m["bias"] = biases[h]
        in_maps.append(m)
    return in_maps


def run(x, W_qkv, b_qkv, W_gate, b_gate, trace=False, **run_kwargs):
    with_bias = bool(np.any(np.asarray(b_qkv)) or np.any(np.asarray(b_gate)))
    nc = _get_nc(with_bias)
    in_maps = _prep_in_maps(x, W_qkv, b_qkv, W_gate, b_gate, with_bias)
    res = run_bass_kernel_spmd(nc, in_maps, list(range(8)), trace=trace, **run_kwargs)
    out = np.empty((B, S, D), dtype=np.float32)
    for core in range(8):
        b, h = core // 2, core % 2
        out[b, :, h * H:(h + 1) * H] = res.results[core]["out"].T
    return out, res


def kernel(x, W_qkv, b_qkv, W_gate, b_gate):
    out, _ = run(x, W_qkv, b_qkv, W_gate, b_gate)
    return out


# revision 17
# speedup vs baseline: 1.2593x; 1.0185x over previous
"""Trainium2 Bass kernel for nn_LinearLatentKernel_84834194031187.

Computes, for x:[B,S,D], W_qkv:[3D,D], W_gate:[D,D] (fp32):
    qkv = x @ W_qkv.T + b_qkv ; q,k,v = split(qkv)
    kv_state = cumsum(k*v, axis=seq)
    out = q * kv_state * sigmoid(x @ W_gate.T + b_gate)

Sharding: 8 cores = (batch b in 0..3) x (channel half h in 0..1); each core
computes out[b, :, h*512:(h+1)*512].

Layout: everything is computed TRANSPOSED, [channel, seq], so that
  - the projection matmuls take host-pretransposed x^T [D,S] directly as the
    moving operand (no PE transposes at all): ps[c,s] = wt[d,c]^T @ x^T[d,s]
  - the cumsum along seq becomes a FREE-dim prefix scan on the Vector engine
    (tensor_tensor_scan, chained across 512-col chunks via initial=prev[:,-1:])
so the Tensor engine runs ONLY the 1024 projection matmuls (8 kt x 4 types x
4 groups x 8 seq chunks, each [128x128]x[128x512] fp16 = 512 cycles), which is
the compute floor. Channel dim per core = 512 -> 4 groups of 128 partitions;
groups alternate between two sets of 4 PSUM banks (8 banks total), and each
type's consumers (sigmoid / copy / kv-mul+scan / final muls) fire as soon as
that type's 8-matmul accumulation completes, so PSUM recycling never stalls
the PE. Output is written transposed [512, S] and untransposed on the host.
"""

import numpy as np

import concourse.bass as bass
import concourse.bacc as bacc
import concourse.tile as tile
import concourse.mybir as mybir
from concourse.bass_utils import run_bass_kernel_spmd

B, S, D = 4, 4096, 1024
H = 512          # output channels per core (half of D)
P = 128
KT = D // P      # 8 contraction tiles
FD = 512         # seq columns per chunk (= PSUM bank capacity in fp32)
NJ = S // FD     # 8 seq chunks
NG = H // P      # 4 channel groups of 128
C = 4 * H        # 2048 projection columns per core (g,k,v,q per group)

f32 = mybir.dt.float32
f16 = mybir.dt.float16

# types within a group, in matmul order: gate first (longest consumer chain
# start), then k, v (kv-mul + scan), then q (final muls + store)
T_G, T_K, T_V, T_Q = 0, 1, 2, 3

_NC_CACHE = {}


def _build(with_bias: bool):
    nc = bacc.Bacc("TRN2", target_bir_lowering=False)

    # host supplies x^T and W^T pre-packed so one chunk (resp. weight group)
    # is contiguous per partition (8KB lines): descriptor generation on the
    # sync queue is ~4ns/descriptor serial, so a 1MB transfer must be 128
    # descriptors, not 1024 -- otherwise the PE stalls at startup
    xT_d = nc.dram_tensor("xT", [NJ, P, KT, FD], f16, kind="ExternalInput")
    wt_d = nc.dram_tensor("wt", [NG, P, KT, 4 * P], f16, kind="ExternalInput")
    if with_bias:
        bias_d = nc.dram_tensor("bias", [P, 4 * NG], f32, kind="ExternalInput")
    out_d = nc.dram_tensor("out", [H, S], f32, kind="ExternalOutput")

    sig = mybir.ActivationFunctionType.Sigmoid
    ident = mybir.ActivationFunctionType.Identity
    mult = mybir.AluOpType.mult
    add = mybir.AluOpType.add

    with tile.TileContext(nc) as tc:
        with (
            tc.tile_pool(name="consts", bufs=1) as consts,
            tc.tile_pool(name="xin", bufs=2) as xin,
            tc.tile_pool(name="work", bufs=1) as work,
            tc.tile_pool(name="scanp", bufs=2) as scanp,
            tc.tile_pool(name="outp", bufs=3) as outp,
            tc.tile_pool(name="psp", bufs=1, space="PSUM") as psp,
        ):
            GC = 4 * P  # 512 columns per group
            # ~8 dummy matmuls on uninitialized SBUF: keeps the PE busy for
            # the ~3.4us HAM activity window while the first DMAs land, so
            # the real matmul stream starts at 2.4GHz instead of 1.2GHz
            warm_w = consts.tile([P, P], f16, tag="warm_w")
            warm_x = consts.tile([P, FD], f16, tag="warm_x")
            nc.gpsimd.memset(warm_w[:], 0.0)
            nc.gpsimd.memset(warm_x[:], 0.0)
            warm_ps = psp.tile([P, FD], f32, tag="ps0_0", name="warm_ps")
            for _ in range(8):
                nc.tensor.matmul(warm_ps[:], warm_w[:], warm_x[:],
                                 start=True, stop=True)

            # staged startup DMAs: (group0, chunk0) land in kt-pair pieces so
            # the first real matmuls start as soon as kt0-1 are in
            wtg_sb = [
                consts.tile([P, KT, GC], f16, tag=f"wtg{gi}", name=f"wtg{gi}")
                for gi in range(NG)
            ]
            xts0 = xin.tile([P, KT, FD], f16, tag="x", name="xts0")
            for lo, hi in ((0, 2), (2, 4), (4, 8)):
                nc.sync.dma_start(wtg_sb[0][:, lo:hi, :], wt_d[0, :, lo:hi, :])
                nc.sync.dma_start(xts0[:, lo:hi, :], xT_d[0, :, lo:hi, :])
            ones_sb = consts.tile([P, FD], f32, tag="ones")
            nc.vector.memset(ones_sb[:], 1.0)
            if with_bias:
                bias_sb = consts.tile([P, 4 * NG], f32, tag="bias")
                nc.sync.dma_start(bias_sb[:], bias_d[:])

            prev_scan = {}
            for j in range(NJ):
                if j == 0:
                    xts = xts0
                else:
                    xts = xin.tile([P, KT, FD], f16, tag="x", name="xts")
                    nc.sync.dma_start(xts[:], xT_d[j])

                for i in range(NG):
                    par = i % 2
                    ps = [
                        psp.tile([P, FD], f32, tag=f"ps{t}_{par}",
                                 name=f"ps{t}_{par}")
                        for t in range(4)
                    ]
                    bcol = (
                        (lambda t: bias_sb[:, (i * 4 + t):(i * 4 + t) + 1])
                        if with_bias else None
                    )
                    g_sb = k_sb = kv_sb = sc = gs_sb = None
                    for t in range(4):
                        col0 = t * P
                        for kt in range(KT):
                            nc.tensor.matmul(
                                ps[t][:],
                                wtg_sb[i][:, kt, col0:col0 + P],
                                xts[:, kt, :],
                                start=(kt == 0),
                                stop=(kt == KT - 1),
                            )
                        # consumers fire as soon as this type's accumulation
                        # is complete, staggered across the group's matmuls
                        if t == T_G:
                            g_sb = work.tile([P, FD], f32, tag=f"g{par}")
                            nc.scalar.activation(
                                g_sb[:], ps[T_G][:], sig,
                                bias=bcol(T_G) if with_bias else 0.0,
                            )
                        elif t == T_K:
                            k_sb = work.tile([P, FD], f32, tag=f"k{par}")
                            if with_bias:
                                nc.scalar.activation(
                                    k_sb[:], ps[T_K][:], ident, bias=bcol(T_K)
                                )
                            else:
                                nc.scalar.copy(k_sb[:], ps[T_K][:])
                        elif t == T_V:
                            kv_sb = work.tile([P, FD], f32, tag=f"kv{par}")
                            if with_bias:
                                nc.vector.scalar_tensor_tensor(
                                    kv_sb[:], ps[T_V][:], bcol(T_V), k_sb[:],
                                    add, mult,
                                )
                            else:
                                nc.vector.tensor_mul(
                                    out=kv_sb[:], in0=k_sb[:], in1=ps[T_V][:]
                                )
                            sc = scanp.tile([P, FD], f32, tag=f"scan{i}")
                            init = 0.0 if j == 0 else prev_scan[i][:, FD - 1:FD]
                            # state = (ones * state) + kv ; out[t] = state
                            nc.vector.tensor_tensor_scan(
                                sc[:], ones_sb[:], kv_sb[:], init, mult, add
                            )
                            prev_scan[i] = sc
                            # gs = sigmoid(g) * kv_state, computed during the
                            # q matmuls so only ONE mul remains after they end
                            gs_sb = work.tile([P, FD], f32, tag=f"gs{par}")
                            nc.vector.tensor_mul(
                                out=gs_sb[:], in0=g_sb[:], in1=sc[:]
                            )
                        else:  # T_Q
                            ob = outp.tile([P, FD], f32, tag="ob")
                            if with_bias:
                                nc.vector.scalar_tensor_tensor(
                                    ob[:], ps[T_Q][:], bcol(T_Q), gs_sb[:],
                                    add, mult,
                                )
                            else:
                                nc.vector.tensor_mul(
                                    out=ob[:], in0=gs_sb[:], in1=ps[T_Q][:]
                                )
                            nc.sync.dma_start(
                                out_d[i * P:(i + 1) * P, j * FD:(j + 1) * FD],
                                ob[:],
                            )
                    if j == 0 and i + 1 < NG:
                        # weight groups 1-3 stream in behind group i's compute
                        nc.sync.dma_start(wtg_sb[i + 1][:], wt_d[i + 1])

    nc.compile()
    return nc


def _get_nc(with_bias: bool):
    if with_bias not in _NC_CACHE:
        _NC_CACHE[with_bias] = _build(with_bias)
    return _NC_CACHE[with_bias]


def _prep_in_maps(x, W_qkv, b_qkv, W_gate, b_gate, with_bias):
    x = np.asarray(x, dtype=np.float32)
    W_qkv = np.asarray(W_qkv, dtype=np.float32)
    W_gate = np.asarray(W_gate, dtype=np.float32)

    # x^T packed [chunk j, partition p, kt, col c] = x[j*FD+c, kt*128+p]
    # so one chunk's DMA is 8KB-contiguous per partition (128 descriptors)
    xTs = [
        np.ascontiguousarray(
            x[b].astype(np.float16).reshape(NJ, FD, KT, P).transpose(0, 3, 2, 1)
        )
        for b in range(B)
    ]

    # weight rows per (group, type): [g_i | k_i | v_i | q_i] blocks of 128
    wts, biases = [], []
    for h in range(2):
        blocks, bcols = [], []
        for i in range(NG):
            r0 = h * H + i * P
            rows = [
                W_gate[r0:r0 + P],
                W_qkv[D + r0:D + r0 + P],
                W_qkv[2 * D + r0:2 * D + r0 + P],
                W_qkv[r0:r0 + P],
            ]
            blocks.extend(rows)
            if with_bias:
                bq = np.asarray(b_qkv, dtype=np.float32)
                bg = np.asarray(b_gate, dtype=np.float32)
                bcols.extend([
                    bg[r0:r0 + P],
                    bq[D + r0:D + r0 + P],
                    bq[2 * D + r0:2 * D + r0 + P],
                    bq[r0:r0 + P],
                ])
        wt = np.concatenate(blocks, axis=0).T  # [1024, 2048]
        # packed [group gi, partition p, kt, col c] = wt[kt*128+p, gi*512+c]
        wts.append(np.ascontiguousarray(
            wt.astype(np.float16).reshape(KT, P, NG, 4 * P).transpose(2, 1, 0, 3)
        ))
        if with_bias:
            biases.append(np.stack(bcols, axis=1).astype(np.float32))  # [128,16]

    in_maps = []
    for core in range(8):
        b, h = core // 2, core % 2
        m = {"xT": xTs[b], "wt": wts[h]}
        if with_bias:
            m["bias"] = biases[h]
        in_maps.append(m)
    return in_maps


def run(x, W_qkv, b_qkv, W_gate, b_gate, trace=False, **run_kwargs):
    with_bias = bool(np.any(np.asarray(b_qkv)) or np.any(np.asarray(b_gate)))
    nc = _get_nc(with_bias)
    in_maps = _prep_in_maps(x, W_qkv, b_qkv, W_gate, b_gate, with_bias)
    res = run_bass_kernel_spmd(nc, in_maps, list(range(8)), trace=trace, **run_kwargs)
    out = np.empty((B, S, D), dtype=np.float32)
    for core in range(8):
        b, h = core // 2, core % 2
        out[b, :, h * H:(h + 1) * H] = res.results[core]["out"].T
    return out, res


def kernel(x, W_qkv, b_qkv, W_gate, b_gate):
    out, _ = run(x, W_qkv, b_qkv, W_gate, b_gate)
    return out
